# revision 7
# baseline (speedup 1.0000x reference)
"""Cost-volume concatenation kernel for Trainium2 (8 NeuronCores).

Reference (per batch b, disparity index d, i = d + MIN_DISP):
  out[b, d, h, w, 0:C]  = left[b, h, w, :]    if 0 <= w - i < W else 0
  out[b, d, h, w, C:2C] = right[b, h, w-i, :] if 0 <= w - i < W else 0

Sharding: disparity-parallel, interleaved -- core c builds disparities
{8j + c : j in 0..15} for the full [B, H, W] volume.  Interleaving
balances valid-span widths (bytes written) across cores.

SPMD trick: run_bass_kernel_spmd runs ONE program on all 8 cores, so the
per-core offset c cannot appear in any access pattern.  The program is
written for i0 = 8j - 112 and all c-dependence lives in the data:
  * rightp input = right pre-shifted by +c columns, zero-padded to W+8
    columns -- the program's static gather rightp[w - i0] then yields
    right[w - i] with the out-of-range mask applied by the padding.
  * cvec input = per-partition scalars [c, W+c]; the left-half validity
    mask is built on-chip at COLUMN granularity (one value per source
    column x = w - i0): mask[x] = (iota(x) >= c) * (iota(x) < W+c), and
    broadcast over the 16 channels with a stride-0 AP in the per-plane
    multiply.
Each plane writes the union-over-c of valid w-spans; columns inside the
union but outside the core's true span receive exact zeros from the
padding/mask; columns outside the union are never written and rely on
ExternalOutput buffers being pre-zeroed (bass2jax donates zero buffers
to PJRT for exactly this purpose).

Tiles: one disparity plane per SBUF tile, 96 h-rows.  Consecutive planes
are staggered by 32 partitions (even -> rows 0:96, odd -> rows 32:128)
and stored on the two HWDGE rings (sync/scalar): a lone 96-partition DMA
only engages 12 of the 16 SBUF AXI ports (~250 GB/s measured); two
staggered concurrent stores cover all 16 (~330 GB/s measured for 128p).

Startup schedule (v2): the profile showed first-store at ~26us because
cvec + the phase-1 input copies sat behind the slow software DGE (gpsimd
descriptor generation, ~131 GB/s + ~6us spin-up) and a 3.5us full-width
mask build.  Now: cvec + both left copies ride the sync HWDGE ring,
both rightp copies ride the scalar HWDGE ring (arrive ~12-17us), the
iota is gpsimd's first instruction and 16x smaller, and only the
slack-rich b=1 loads stay on the software DGE.  The scalar engine's
store triggers are emitted 3 planes behind its copies so a trigger's
wait on the vector mul never stalls the copy stream.
"""

import os
import sys

sys.path.insert(0, "/opt/trn_rl_repo")

import numpy as np

B, H, W, C = 2, 96, 192, 16
D = 128
MIN_DISP = -112
N_CORES = 8
DPC = D // N_CORES         # 16 disparity planes per core
PAD = 8                    # rightp padded to W + PAD source columns
WP = W + PAD
COLS = W * 2 * C           # 6144 interleaved f32 per (b,d,h) row

_CACHE = {}


def _plane_span(j):
    """Union-over-c valid w-span for plane j (program-static)."""
    i0 = 8 * j + MIN_DISP
    if i0 < 0:
        us, ue = 0, min(W + i0 + (N_CORES - 1), W)
    else:
        us, ue = i0, W
    return i0, us, ue


def _build_program():
    from concourse import bacc, mybir
    import concourse.tile as tile

    nc = bacc.Bacc(
        "TRN2", target_bir_lowering=False, debug=False, num_devices=N_CORES
    )
    f32 = mybir.dt.float32
    left = nc.dram_tensor("left", [B, H, W * C], f32, kind="ExternalInput")
    rightp = nc.dram_tensor("rightp", [B, H, WP * C], f32, kind="ExternalInput")
    cvec = nc.dram_tensor("cvec", [128, 2], f32, kind="ExternalInput")
    out = nc.dram_tensor("out", [B, DPC, H, COLS], f32, kind="ExternalOutput")

    with tile.TileContext(nc) as tc:
        with (
            tc.tile_pool(name="inputs", bufs=1) as ipool,
            tc.tile_pool(name="work", bufs=4) as wpool,
        ):
            # Input tiles, two stagger phases: phase 0 data at rows 0:96,
            # phase 1 at rows 32:128.
            lsb = {}   # (b, phase) -> (tile, row0)
            rsb = {}
            for b in range(B):
                for ph in range(2):
                    r0 = 32 * ph
                    lt = ipool.tile([128, W * C], f32, tag=f"l{b}{ph}")
                    rt = ipool.tile([128, WP * C], f32, tag=f"r{b}{ph}")
                    lsb[(b, ph)] = (lt, r0)
                    rsb[(b, ph)] = (rt, r0)

            cv = ipool.tile([128, 2], f32, tag="cvec")
            msk = ipool.tile([128, WP], f32, tag="msk")
            xio = ipool.tile([128, WP], f32, tag="xio")

            # cvec rides the software DGE first (tiny; lands ~13us, same
            # time the mask consumer needs it).  Putting it at the head
            # of a HWDGE ring instead costs a ~6us per-DMA bubble in
            # front of the critical l00/r00 loads (measured).
            nc.gpsimd.dma_start(cv[:, :], cvec.ap())
            nc.gpsimd.iota(
                xio[:, :], [[1, WP]], channel_multiplier=0,
                allow_small_or_imprecise_dtypes=True,
            )
            # HWDGE rings carry only the two phase-0 b=0 inputs so the
            # store stream starts right behind them (~16us); single-queue
            # HWDGE runs ~160 GB/s, so anything more at the head delays
            # the first stores.
            nc.sync.dma_start(lsb[(0, 0)][0][0:96, :], left.ap()[0])
            nc.scalar.dma_start(rsb[(0, 0)][0][0:96, :], rightp.ap()[0])
            nc.gpsimd.dma_start(lsb[(0, 1)][0][32:128, :], left.ap()[0])
            nc.gpsimd.dma_start(rsb[(0, 1)][0][32:128, :], rightp.ap()[0])
            for b2 in range(1, B):
                nc.gpsimd.dma_start(lsb[(b2, 0)][0][0:96, :], left.ap()[b2])
                nc.gpsimd.dma_start(rsb[(b2, 0)][0][0:96, :], rightp.ap()[b2])
                nc.gpsimd.dma_start(lsb[(b2, 1)][0][32:128, :], left.ap()[b2])
                nc.gpsimd.dma_start(rsb[(b2, 1)][0][32:128, :], rightp.ap()[b2])

            # Column-granular mask over source columns x = w - i0,
            # identical on every partition: 1.0 iff c <= x < W + c.
            nc.vector.tensor_single_scalar(
                msk[:, :], xio[:, :], cv[:, 0:1], mybir.AluOpType.is_ge
            )
            nc.vector.tensor_single_scalar(
                xio[:, :], xio[:, :], cv[:, 1:2], mybir.AluOpType.is_lt
            )
            nc.vector.tensor_mul(msk[:, :], msk[:, :], xio[:, :])

            n_planes = B * DPC
            # The first PREFIX planes all use phase 0 (rows 0:96): their
            # inputs (l00/r00 on the HWDGE rings) land ~16us, while the
            # phase-1 copies ride the slow software DGE and only land
            # ~30us.  Storing the prefix alternately on BOTH rings keeps
            # both busy from ~25us (at the 12-port ~250 GB/s same-rows
            # rate); staggered alternation resumes once phase-1 exists.
            PREFIX = 4
            # Plane order is width-aware.  Spans grow monotonically with
            # j (87..192 cols), so the natural order piles the four
            # widest stores (2.3-3 MB, 14-18us drains) at the batch
            # boundary; with only 4 staging slots the slot-recycle waits
            # on those long drains starve compute there (measured 175
            # GB/s dip).  Instead: narrow prefix (fast recycle while the
            # rings share ports), widest planes mid-batch, narrow again
            # at the end so the cross-batch handoff and the final drain
            # are short.
            order0 = [0, 1, 2, 3, 14, 15, 12, 13, 10, 11, 8, 9, 6, 7, 4, 5]
            order1 = [0, 1, 14, 15, 12, 13, 10, 11, 8, 9, 6, 7, 4, 5, 2, 3]
            seq = [(0, j) for j in order0] + [(1, j) for j in order1]

            for n in range(n_planes):
                b, j = seq[n]
                ph = 0 if n < PREFIX else n % 2
                i0, us, ue = _plane_span(j)
                nw = ue - us
                x0 = us - i0      # source column offset into rightp/mask

                lt, r0 = lsb[(b, ph)]
                rt, _ = rsb[(b, ph)]
                T = wpool.tile([128, COLS], f32, tag="out")
                # Compute-engine APs must start in a naturally-aligned
                # partition block, so the 32-offset phase runs one full
                # [0:128) op: rows 0:32 compute garbage from never-
                # written input rows, but are never stored.  Same wall
                # time as a 96-row op (time ~ free size, lanes are
                # parallel), vs 2x for a [32:64)+[64:128) split.
                segs = [(0, 128)] if r0 == 32 else [(0, 96)]
                for s0, sn in segs:
                    s1 = s0 + sn
                    t_chunk = T[s0:s1, us * 32 : ue * 32].rearrange(
                        "p (w c) -> p w c", c=32
                    )
                    src_r = rt[s0:s1, x0 * C : (x0 + nw) * C].rearrange(
                        "p (w c) -> p w c", c=C
                    )
                    src_l = lt[s0:s1, us * C : ue * C].rearrange(
                        "p (w c) -> p w c", c=C
                    )
                    src_m = (
                        msk[s0:s1, x0 : x0 + nw]
                        .unsqueeze(2)
                        .broadcast_to([sn, nw, C])
                    )
                    nc.scalar.copy(t_chunk[:, :, C : 2 * C], src_r)
                    nc.vector.tensor_mul(t_chunk[:, :, 0:C], src_l, src_m)

                dst = out.ap()[b, j, :, us * 32 : ue * 32]
                src = T[r0 : r0 + H, us * 32 : ue * 32]
                # Ring alternates by plane index (not phase) so both
                # rings flow during the phase-0-only prefix.
                store_engine = nc.sync if n % 2 == 0 else nc.scalar
                store_engine.dma_start(dst, src)

    nc.compile()
    return nc


def _get_program():
    if "nc" not in _CACHE:
        _CACHE["nc"] = _build_program()
    return _CACHE["nc"]


def kernel(left, right):
    from concourse.bass_utils import run_bass_kernel_spmd

    left = np.ascontiguousarray(left, dtype=np.float32).reshape(B, H, W * C)
    right = np.ascontiguousarray(right, dtype=np.float32)
    nc = _get_program()

    in_maps = []
    for c in range(N_CORES):
        rp = np.zeros((B, H, WP, C), dtype=np.float32)
        rp[:, :, c : c + W] = right
        cv = np.empty((128, 2), dtype=np.float32)
        cv[:, 0] = float(c)
        cv[:, 1] = float(W + c)
        in_maps.append(
            {
                "left": left,
                "rightp": rp.reshape(B, H, WP * C),
                "cvec": cv,
            }
        )

    prof_dir = os.environ.get("BASS_NTFF_DIR")
    if prof_dir:
        from trn_agent_boot.trn_boot import _ntff_profile_via_ctypes

        hook = _ntff_profile_via_ctypes("/opt/axon/libaxon_pjrt.so")
        with hook(prof_dir, [0]):
            res = run_bass_kernel_spmd(nc, in_maps, core_ids=list(range(N_CORES)))
    else:
        res = run_bass_kernel_spmd(nc, in_maps, core_ids=list(range(N_CORES)))

    # parts[c][b, j] is disparity d = 8j + c -> stack on a new axis after j.
    parts = [
        res.results[c]["out"].reshape(B, DPC, H, W, 2 * C)
        for c in range(N_CORES)
    ]
    return np.stack(parts, axis=2).reshape(B, D, H, W, 2 * C)
